# revision 23
# baseline (speedup 1.0000x reference)
"""Trainium2 Bass kernel for nn_CC_DC_and_CE_loss (segment_reduce).

Strategy (v2)
-------------
loss = global DC+CE + per-connected-component term.  Inputs carry a
structured Voronoi partition: ``vor`` is a fixed 2x2x4 block grid (ids
1..16) and ``lbl = where(target != 0, vor, 0)``.  Verified on host
(exact integer compares); if the check fails we fall back to exact
numpy.  Under the structure all 17-bin segmented reductions collapse to
block sums over the 16 cells.

Device work per voxel (channel order [2,3,1,0], all bf16):
  e = exp(o); s = sum_c e_c (two pair-adds); lns = Ln(s);
  rr = Exp(-lns); p_c = e_c*rr for c in {2,3,1}; pm_c = p_c*m_c
  (one 3F op); f1 = p1*m0 (GpSimd).  The CE map is ce = lns - o_tgt
  where the o_tgt block sums are an exact host-side target-indexed
  gather (ints decide, host stages) - this removes the whole p_tgt/
  Ln(p_tgt) chain of the previous version.  lns doubles as the CE
  reduction stream.
PE reduces 8 maps per group via ones-stationary pair matmuls into 4
accumulating PSUM streams: (p2,p3) (p1,f1) (pm2,pm3) (pm1,lns); y-half
predicates in the stationary + x kept in PSUM columns give the 16-cell
resolution; the host folds and evaluates the final formula in f64.

Sharding: data-parallel over (batch, z): core i handles sample i//4,
z-slabs [32*(i%4), 32*(i%4+1)), groups [4,8,8,8,4].  Group layout:
partition p = (z_local, y//gs), col f = (y%gs)*128 + x, so DMAs are
long contiguous runs in the host-staged buffers; y-half (by) is a
partition predicate; x = f%128 keeps the 4 x-blocks separable in PSUM.
Pipeline: ACT queue [exp(g), Ln(g-1), rExp(g-1)] so the exp of the
next group covers the s-sum latency; DVE [qa(g), st(g), p(g-1)x3,
pm(g-1)]; GpSimd does f1(g-1); PE streams lag one group.
"""

import sys

sys.path.insert(0, "/opt/trn_rl_repo")

import numpy as np

B, C, D = 2, 4, 128
NCC = 16
SMOOTH = 1e-5
ZSH = 32          # z-slabs per core
GROUPS = [4, 4, 8, 8, 8]
NCORES = 8
CPERM = [2, 3, 1, 0]   # channel order of the staged logits / masks

_cache = {}


def _build_program():
    import concourse.bacc as bacc
    import concourse.tile as tile
    import concourse.mybir as mybir

    # Pin every activation to the one table set holding BOTH exp and ln, so
    # the exp->ln->exp chain doesn't thrash ACT_TABLE_LOADs (~1.3us each).
    if not getattr(bacc, "_act_tables_pinned", False):
        _orig_get_tables = bacc.get_activation_tables

        def _pinned_tables(arch):
            tables = _orig_get_tables(arch)
            return {
                name: (funcs if name == "natural_log_exp_and_others" else set())
                for name, funcs in tables.items()
            }

        bacc.get_activation_tables = _pinned_tables
        bacc._act_tables_pinned = True

    AF = mybir.ActivationFunctionType
    ALU = mybir.AluOpType
    dt = mybir.dt

    nc = bacc.Bacc("TRN2", num_devices=NCORES)

    o_dram = nc.dram_tensor("o", [C, ZSH, D, D], dt.bfloat16, kind="ExternalInput")
    m_dram = nc.dram_tensor("m", [C, ZSH, D, D], dt.bfloat16, kind="ExternalInput")
    # hv y-half ones: cols 0,1 for gs=8 (p%16<8); 2,3 for gs=4; 4,5 for gs=2
    hv_dram = nc.dram_tensor("hv", [128, 6], dt.bfloat16, kind="ExternalInput")
    res_dram = nc.dram_tensor("res", [2, 2048], dt.float32, kind="ExternalOutput")

    with tile.TileContext(nc) as tc:
        with (
            tc.tile_pool(name="work", bufs=2) as work,
            tc.tile_pool(name="inp", bufs=3) as inp,
            tc.tile_pool(name="minp", bufs=4) as minp,
            tc.tile_pool(name="work3", bufs=3) as work3,
            tc.tile_pool(name="const", bufs=1) as constp,
            tc.tile_pool(name="psum", bufs=1, space="PSUM") as psum,
            tc.tile_pool(name="outp", bufs=1) as outp,
        ):
            halves = constp.tile([128, 6], dt.bfloat16, tag="halves", name="halves")

            # psum streams: [0:512) (p2,p3) | [512:1024) (p1,f1)
            #               [1024:1536) (pm2,pm3) | [1536:2048) (pm1,lns)
            ps = psum.tile([2, 2048], dt.float32, tag="ps", name="ps")

            def pair_mm(dst0, rhs2F, F, lhs, first, last):
                rhs3 = rhs2F.rearrange("p (a f) -> p a f", a=2)
                nj = F // 256
                for j in range(nj):
                    nc.tensor.matmul(
                        ps[:, dst0 : dst0 + 512],
                        lhs,
                        rhs3[:, :, 256 * j : 256 * (j + 1)],
                        start=(first and j == 0),
                        stop=(last and j == nj - 1),
                    )

            def st_dma_o(cur):
                GS, z0 = cur["GS"], cur["z0"]
                nc.sync.dma_start(
                    cur["obig"][:].rearrange("p (c f) -> p c f", c=C),
                    o_dram[:, z0 : z0 + GS]
                    .rearrange("c z y x -> c (z y x)")
                    .rearrange("c (p f) -> p c f", p=128),
                )

            def st_dma_m(cur):
                GS, z0 = cur["GS"], cur["z0"]
                nc.sync.dma_start(
                    cur["MQ"][:].rearrange("p (c f) -> p c f", c=C),
                    m_dram[:, z0 : z0 + GS]
                    .rearrange("c z y x -> c (z y x)")
                    .rearrange("c (p f) -> p c f", p=128),
                )

            def st_exp(cur):
                nc.scalar.activation(cur["ebig"][:], cur["obig"][:], AF.Exp)

            def st_sum(cur):
                # s = (e2+e1) + (e3+e0), both pair-adds on DVE
                F = cur["F"]
                nc.vector.tensor_tensor(cur["qa"][:], cur["ebig"][:, 0 : 2 * F],
                                        cur["ebig"][:, 2 * F : 4 * F], ALU.add)
                nc.vector.tensor_tensor(cur["st"][:], cur["qa"][:, 0:F],
                                        cur["qa"][:, F : 2 * F], ALU.add)

            def st_act_ln(pg):
                # lns -> PMX[3F:4F] (doubles as the CE stream); rr = 1/s
                F = pg["F"]
                nc.scalar.activation(pg["PMX"][:, 3 * F : 4 * F], pg["st"][:],
                                     AF.Ln)
                nc.scalar.activation(pg["rr"][:], pg["PMX"][:, 3 * F : 4 * F],
                                     AF.Exp, scale=-1.0)


            def st_pe_cd(pg, plast):
                # streams C=(pm2,pm3), D=(pm1,lns) of group g
                F, lhs = pg["F"], pg["lhs"]
                pair_mm(1024, pg["PMX"][:, 0 : 2 * F], F, lhs,
                        pg["first"], plast)
                pair_mm(1536, pg["PMX"][:, 2 * F : 4 * F], F, lhs,
                        pg["first"], plast)

            def st_products(pg, ba_last, prev):
                # DVE products of group g; PE issue staggered so the PE
                # alternates (C,D)(g-1) then (B,A)(g) with every wait already
                # satisfied when its matmul reaches the queue head - the PE
                # stays gapless and ramps to the full 2.4 GHz p-state.
                F = pg["F"]
                PQX, PMX, MQ = pg["PQX"], pg["PMX"], pg["MQ"]
                nc.vector.tensor_tensor(
                    PQX[:, 2 * F : 3 * F], pg["ebig"][:, 2 * F : 3 * F],
                    pg["rr"][:], ALU.mult)
                nc.vector.tensor_tensor(
                    PQX[:, 3 * F : 4 * F], PQX[:, 2 * F : 3 * F],
                    MQ[:, 3 * F : 4 * F], ALU.mult)
                if prev is not None:
                    st_pe_cd(prev, False)
                pair_mm(512, PQX[:, 2 * F : 4 * F], F, pg["lhs"],
                        pg["first"], ba_last)
                for c in range(2):
                    nc.vector.tensor_tensor(
                        PQX[:, c * F : (c + 1) * F],
                        pg["ebig"][:, c * F : (c + 1) * F], pg["rr"][:],
                        ALU.mult)
                pair_mm(0, PQX[:, 0 : 2 * F], F, pg["lhs"],
                        pg["first"], ba_last)
                nc.vector.tensor_tensor(
                    PMX[:, 0 : 3 * F], PQX[:, 0 : 3 * F], MQ[:, 0 : 3 * F],
                    ALU.mult)

            def mk(g, GS, z0):
                F = GS * D
                lhs = {8: halves[:, 0:2], 4: halves[:, 2:4],
                       2: halves[:, 4:6]}[GS]
                return {
                    "F": F, "GS": GS, "z0": z0, "first": g == 0, "lhs": lhs,
                    "obig": inp.tile([128, 4 * F], dt.bfloat16, tag="obig",
                                     name="obig"),
                    "ebig": inp.tile([128, 4 * F], dt.bfloat16, tag="ebig",
                                     name="ebig"),
                    "MQ": minp.tile([128, 4 * F], dt.bfloat16, tag="MQ",
                                    name="MQ"),
                    "qa": work.tile([128, 2 * F], dt.bfloat16, tag="qa",
                                    name="qa"),
                    "st": work.tile([128, F], dt.bfloat16, tag="st", name="st"),
                    "rr": work.tile([128, F], dt.bfloat16, tag="rr", name="rr"),
                    "PQX": work3.tile([128, 4 * F], dt.bfloat16, tag="PQX",
                                      name="PQX"),
                    "PMX": work3.tile([128, 4 * F], dt.bfloat16, tag="PMX",
                                      name="PMX"),
                }

            G = len(GROUPS)
            gl = [None] * G
            z0 = 0
            for g, GS in enumerate(GROUPS):
                gl[g] = mk(g, GS, z0)
                z0 += GS
            st_dma_o(gl[0])
            st_dma_o(gl[1])
            st_dma_m(gl[0])
            nc.sync.dma_start(halves[:], hv_dram[:])
            # 2-deep software pipeline; per period k:
            #   ACT [exp(k), ln(k-1), rexp(k-1)]
            #   DVE [qa(k-1), st(k-1), products(k-2)]
            #   PE  [streams(k-2)] (interleaved with products)
            # No queue-head ever waits on work issued later in the same
            # period on another engine (except ln on st, which the 4F exp
            # covers).  o-DMAs run one group ahead of m-DMAs.
            for k in range(G + 2):
                if k + 2 < G:
                    st_dma_o(gl[k + 2])
                if k + 1 < G:
                    st_dma_m(gl[k + 1])
                if k < G:
                    st_exp(gl[k])
                if 1 <= k <= G:
                    st_sum(gl[k - 1])
                    st_act_ln(gl[k - 1])
                if 2 <= k <= G + 1:
                    g = k - 2
                    st_products(gl[g], g == G - 1,
                                gl[g - 1] if g >= 1 else None)
            st_pe_cd(gl[G - 1], True)

            # drain: psum -> sbuf split across two engines, then DMA out
            ob = outp.tile([2, 2048], dt.float32, tag="ob", name="ob")
            nc.vector.tensor_scalar(ob[:, 0:1024], ps[:, 0:1024], 0.0, None,
                                    mybir.AluOpType.add)
            nc.scalar.copy(ob[:, 1024:2048], ps[:, 1024:2048])
            nc.sync.dma_start(res_dram[:], ob[:])

    nc.compile()
    return nc


def _get_program():
    if "nc" not in _cache:
        _cache["nc"] = _build_program()
    return _cache["nc"]


def _is_structured(out, target, lbl, vor, n_cc):
    try:
        if int(n_cc) != NCC:
            return False
        if out.shape != (B, C, D, D, D) or target.shape != (B, 1, D, D, D):
            return False
        if lbl.shape != (B, D, D, D) or vor.shape != (B, D, D, D):
            return False
        bz = np.arange(D) // (D // 2)
        bx = np.arange(D) // (D // 4)
        grid = (bz[:, None, None] * 8 + bz[None, :, None] * 4 + bx[None, None, :] + 1)
        if not (vor == grid[None].astype(vor.dtype)).all():
            return False
        if not (lbl == np.where(target[:, 0] != 0, vor, 0).astype(lbl.dtype)).all():
            return False
        return True
    except Exception:
        return False


def _halves_np():
    import ml_dtypes

    hv = np.zeros((128, 6), dtype=ml_dtypes.bfloat16)
    p = np.arange(128)
    hv[(p % 16) < 8, 0] = 1
    hv[(p % 16) >= 8, 1] = 1
    hv[(p % 32) < 16, 2] = 1
    hv[(p % 32) >= 16, 3] = 1
    hv[(p % 64) < 32, 4] = 1
    hv[(p % 64) >= 32, 5] = 1
    return hv


def run_device(out, target, trace=False, trace_cores=None):
    """Run the 8-core device program; returns (per-core res arrays, results)."""
    import ml_dtypes
    from concourse.bass_utils import run_bass_kernel_spmd

    nc = _get_program()
    bf16 = ml_dtypes.bfloat16
    hv = _halves_np()
    # stage permuted-channel bf16 logits and one-hot masks once per sample
    operm = {}
    mperm = {}
    for b in range(B):
        operm[b] = np.ascontiguousarray(out[b, CPERM]).astype(bf16)
        oh = (target[b, 0, None] == np.array(CPERM)[:, None, None, None])
        mperm[b] = oh.astype(bf16)
    in_maps = []
    for i in range(NCORES):
        b, z0 = i // 4, ZSH * (i % 4)
        in_maps.append({
            "o": np.ascontiguousarray(operm[b][:, z0 : z0 + ZSH]),
            "m": np.ascontiguousarray(mperm[b][:, z0 : z0 + ZSH]),
            "hv": hv,
        })
    results = run_bass_kernel_spmd(
        nc, in_maps, core_ids=list(range(NCORES)), trace=trace,
        trace_cores=trace_cores,
    )
    return [results.results[i]["res"] for i in range(NCORES)], results


def _combine(res_list, out, target):
    """Host combine of per-core partial sums + exact target-derived counts."""
    N = D ** 3
    tgt = target[:, 0].astype(np.int64)

    cnt = np.zeros((B, C))
    fgb = np.zeros((B, 16))           # foreground voxels per Voronoi cell
    OTb = np.zeros((B, 16))           # exact block sums of o_tgt (f64, host)
    for b in range(B):
        cnt[b] = np.bincount(tgt[b].ravel(), minlength=C)[:C]
        fg = (tgt[b] != 0).reshape(2, 64, 2, 64, 4, 32)
        fgb[b] = fg.sum(axis=(1, 3, 5)).reshape(16)
        ot = np.take_along_axis(out[b], tgt[b][None], axis=0)[0].astype(np.float64)
        OTb[b] = ot.reshape(2, 64, 2, 64, 4, 32).sum(axis=(1, 3, 5)).reshape(16)

    P1 = np.zeros((B, 2, 2, 128))     # [b, bz, by, x] block partials of p1
    F1 = np.zeros((B, 2, 2, 128))     # ... of p1*(t==0)
    LS = np.zeros((B, 2, 2, 128))     # ... of ln(s)
    Sp = np.zeros((B, 3))             # global sums of p1, p2, p3
    tp = np.zeros((B, 3))             # global sums of pm1, pm2, pm3

    def fold(region):                 # [2, 256] -> [yhalf, x]
        return region.reshape(2, 2, 128).sum(axis=1)

    for i in range(NCORES):
        b, bz = i // 4, (i % 4) // 2
        r = res_list[i].astype(np.float64)
        Sp[b, 1] += r[:, 0:256].sum()
        Sp[b, 2] += r[:, 256:512].sum()
        P1[b, bz] += fold(r[:, 512:768])
        F1[b, bz] += fold(r[:, 768:1024])
        tp[b, 1] += r[:, 1024:1280].sum()
        tp[b, 2] += r[:, 1280:1536].sum()
        tp[b, 0] += r[:, 1536:1792].sum()
        LS[b, bz] += fold(r[:, 1792:2048])
    Sp[:, 0] = P1.sum(axis=(1, 2, 3))

    def blocks(arr):  # [b, bz, by, x] -> [b, 16] cells (bz*8 + by*4 + x//32)
        return arr.reshape(B, 2, 2, 4, 32).sum(axis=-1).reshape(B, 16)

    Pb, Fb, Lb = blocks(P1), blocks(F1), blocks(LS)

    # ---- global DC_and_CE (ce = lns - o_tgt) ----
    ce_global = (LS.sum() - OTb.sum()) / (B * N)
    fp = Sp - tp
    fn = cnt[:, 1:] - tp
    dc = (2.0 * tp + SMOOTH) / np.maximum(2.0 * tp + fp + fn + SMOOTH, 1e-8)
    global_loss = ce_global - dc.mean()

    # ---- per-component term ----
    cnt_block = float((D // 2) * (D // 2) * (D // 4))
    A = Pb - Fb                      # tp_c
    fn_c = fgb - A
    fp_c = Fb
    dc_c = (2.0 * A + SMOOTH) / np.maximum(2.0 * A + fn_c + fp_c + SMOOTH, 1e-8)
    ce_t = (Lb - OTb) / cnt_block
    cc_term = (-dc_c + ce_t).mean()

    return np.float32(global_loss + cc_term)


def _reference_numpy(out, target, lbl, vor, n_cc):
    """Exact fallback for arbitrary inputs (mirrors reference.py)."""
    n_cc = int(n_cc)
    o = out.astype(np.float64)
    tgt = target[:, 0].astype(np.int64)
    mx = o.max(axis=1, keepdims=True)
    eo = np.exp(o - mx)
    se = eo.sum(axis=1, keepdims=True)
    logp = o - mx - np.log(se)
    probs = np.exp(logp)
    ce_map = -np.take_along_axis(logp, tgt[:, None], axis=1)[:, 0]

    ce_global = ce_map.mean()
    onehot = (tgt[:, None] == np.arange(C)[None, :, None, None, None]).astype(np.float64)
    ax = (2, 3, 4)
    tp = (probs * onehot).sum(axis=ax)
    fp = (probs * (1.0 - onehot)).sum(axis=ax)
    fn = ((1.0 - probs) * onehot).sum(axis=ax)
    dc = (2.0 * tp + SMOOTH) / np.maximum(2.0 * tp + fp + fn + SMOOTH, 1e-8)
    dice_global = -dc[:, 1:].mean()
    global_loss = ce_global + dice_global

    p1 = probs[:, 1].reshape(B, -1)
    lblf = lbl.reshape(B, -1).astype(np.int64)
    vorf = vor.reshape(B, -1).astype(np.int64)
    cef = ce_map.reshape(B, -1)

    def seg(v, idx):
        outv = np.zeros((B, n_cc + 1))
        for b in range(B):
            outv[b] = np.bincount(idx[b], weights=v[b], minlength=n_cc + 1)[: n_cc + 1]
        return outv

    tp_c = seg(p1, lblf)[:, 1:]
    fn_c = seg(1.0 - p1, lblf)[:, 1:]
    fp_c = seg(p1 * (lblf == 0), vorf)[:, 1:]
    ce_c = seg(cef, vorf)[:, 1:]
    cnt_c = seg(np.ones_like(p1), vorf)[:, 1:]
    dc_c = (2.0 * tp_c + SMOOTH) / np.maximum(2.0 * tp_c + fn_c + fp_c + SMOOTH, 1e-8)
    ce_t = ce_c / np.maximum(cnt_c, 1.0)
    cc_term = (-dc_c + ce_t).mean()
    return np.float32(global_loss + cc_term)


def kernel(out, target, lbl, vor, n_cc):
    if not _is_structured(out, target, lbl, vor, n_cc):
        return _reference_numpy(out, target, lbl, vor, n_cc)
    res_list, _ = run_device(out, target)
    return _combine(res_list, out, target)


if __name__ == "__main__":
    rng = np.random.default_rng(0)
    o = rng.standard_normal((B, C, D, D, D), dtype=np.float32)
    t = rng.integers(0, C, (B, 1, D, D, D)).astype(np.int32)
    bz = np.arange(D) // (D // 2)
    bx = np.arange(D) // (D // 4)
    grid = (bz[:, None, None] * 8 + bz[None, :, None] * 4 + bx[None, None, :] + 1).astype(np.int32)
    v = np.broadcast_to(grid, (B, D, D, D)).copy()
    l = np.where(t[:, 0] != 0, v, 0).astype(np.int32)
    got = kernel(out=o, target=t, lbl=l, vor=v, n_cc=np.int64(16))
    want = _reference_numpy(o, t, l, v, 16)
    print("device:", got, "ref:", want, "rel err:", abs(got - want) / abs(want))


# revision 24
# speedup vs baseline: 1.0164x; 1.0164x over previous
"""Trainium2 Bass kernel for nn_CC_DC_and_CE_loss (segment_reduce).

Strategy (v2)
-------------
loss = global DC+CE + per-connected-component term.  Inputs carry a
structured Voronoi partition: ``vor`` is a fixed 2x2x4 block grid (ids
1..16) and ``lbl = where(target != 0, vor, 0)``.  Verified on host
(exact integer compares); if the check fails we fall back to exact
numpy.  Under the structure all 17-bin segmented reductions collapse to
block sums over the 16 cells.

Device work per voxel (channel order [2,3,1,0], all bf16):
  e = exp(o); s = sum_c e_c (two pair-adds); lns = Ln(s);
  rr = Exp(-lns); p_c = e_c*rr for c in {2,3,1}; pm_c = p_c*m_c
  (one 3F op); f1 = p1*m0 (GpSimd).  The CE map is ce = lns - o_tgt
  where the o_tgt block sums are an exact host-side target-indexed
  gather (ints decide, host stages) - this removes the whole p_tgt/
  Ln(p_tgt) chain of the previous version.  lns doubles as the CE
  reduction stream.
PE reduces 8 maps per group via ones-stationary pair matmuls into 4
accumulating PSUM streams: (p2,p3) (p1,f1) (pm2,pm3) (pm1,lns); y-half
predicates in the stationary + x kept in PSUM columns give the 16-cell
resolution; the host folds and evaluates the final formula in f64.

Sharding: data-parallel over (batch, z): core i handles sample i//4,
z-slabs [32*(i%4), 32*(i%4+1)), groups [4,8,8,8,4].  Group layout:
partition p = (z_local, y//gs), col f = (y%gs)*128 + x, so DMAs are
long contiguous runs in the host-staged buffers; y-half (by) is a
partition predicate; x = f%128 keeps the 4 x-blocks separable in PSUM.
Pipeline: ACT queue [exp(g), Ln(g-1), rExp(g-1)] so the exp of the
next group covers the s-sum latency; DVE [qa(g), st(g), p(g-1)x3,
pm(g-1)]; GpSimd does f1(g-1); PE streams lag one group.
"""

import sys

sys.path.insert(0, "/opt/trn_rl_repo")

import numpy as np

B, C, D = 2, 4, 128
NCC = 16
SMOOTH = 1e-5
ZSH = 32          # z-slabs per core
GROUPS = [4, 4, 8, 8, 8]
NCORES = 8
CPERM = [2, 3, 1, 0]   # channel order of the staged logits / masks

_cache = {}


def _build_program():
    import concourse.bacc as bacc
    import concourse.tile as tile
    import concourse.mybir as mybir

    # Pin every activation to the one table set holding BOTH exp and ln, so
    # the exp->ln->exp chain doesn't thrash ACT_TABLE_LOADs (~1.3us each).
    if not getattr(bacc, "_act_tables_pinned", False):
        _orig_get_tables = bacc.get_activation_tables

        def _pinned_tables(arch):
            tables = _orig_get_tables(arch)
            return {
                name: (funcs if name == "natural_log_exp_and_others" else set())
                for name, funcs in tables.items()
            }

        bacc.get_activation_tables = _pinned_tables
        bacc._act_tables_pinned = True

    AF = mybir.ActivationFunctionType
    ALU = mybir.AluOpType
    dt = mybir.dt

    nc = bacc.Bacc("TRN2", num_devices=NCORES)

    o_dram = nc.dram_tensor("o", [C, ZSH, D, D], dt.bfloat16, kind="ExternalInput")
    m_dram = nc.dram_tensor("m", [C, ZSH, D, D], dt.bfloat16, kind="ExternalInput")
    # hv y-half ones: cols 0,1 for gs=8 (p%16<8); 2,3 for gs=4; 4,5 for gs=2
    hv_dram = nc.dram_tensor("hv", [128, 6], dt.bfloat16, kind="ExternalInput")
    res_dram = nc.dram_tensor("res", [2, 2048], dt.float32, kind="ExternalOutput")

    with tile.TileContext(nc) as tc:
        with (
            tc.tile_pool(name="work", bufs=2) as work,
            tc.tile_pool(name="inp", bufs=3) as inp,
            tc.tile_pool(name="minp", bufs=4) as minp,
            tc.tile_pool(name="work3", bufs=4) as work3,
            tc.tile_pool(name="const", bufs=1) as constp,
            tc.tile_pool(name="psum", bufs=1, space="PSUM") as psum,
            tc.tile_pool(name="outp", bufs=1) as outp,
        ):
            halves = constp.tile([128, 6], dt.bfloat16, tag="halves", name="halves")

            # psum streams: [0:512) (p2,p3) | [512:1024) (p1,f1)
            #               [1024:1536) (pm2,pm3) | [1536:2048) (pm1,lns)
            ps = psum.tile([2, 2048], dt.float32, tag="ps", name="ps")

            def pair_mm(dst0, rhs2F, F, lhs, first, last):
                rhs3 = rhs2F.rearrange("p (a f) -> p a f", a=2)
                nj = F // 256
                for j in range(nj):
                    nc.tensor.matmul(
                        ps[:, dst0 : dst0 + 512],
                        lhs,
                        rhs3[:, :, 256 * j : 256 * (j + 1)],
                        start=(first and j == 0),
                        stop=(last and j == nj - 1),
                    )

            def st_dma_o(cur):
                GS, z0 = cur["GS"], cur["z0"]
                nc.sync.dma_start(
                    cur["obig"][:].rearrange("p (c f) -> p c f", c=C),
                    o_dram[:, z0 : z0 + GS]
                    .rearrange("c z y x -> c (z y x)")
                    .rearrange("c (p f) -> p c f", p=128),
                )

            def st_dma_m(cur):
                GS, z0 = cur["GS"], cur["z0"]
                nc.sync.dma_start(
                    cur["MQ"][:].rearrange("p (c f) -> p c f", c=C),
                    m_dram[:, z0 : z0 + GS]
                    .rearrange("c z y x -> c (z y x)")
                    .rearrange("c (p f) -> p c f", p=128),
                )

            def st_exp(cur):
                nc.scalar.activation(cur["ebig"][:], cur["obig"][:], AF.Exp)

            def st_sum(cur):
                # s = (e2+e1) + (e3+e0), both pair-adds on DVE
                F = cur["F"]
                nc.vector.tensor_tensor(cur["qa"][:], cur["ebig"][:, 0 : 2 * F],
                                        cur["ebig"][:, 2 * F : 4 * F], ALU.add)
                nc.vector.tensor_tensor(cur["st"][:], cur["qa"][:, 0:F],
                                        cur["qa"][:, F : 2 * F], ALU.add)

            def st_act_ln(pg):
                # lns -> PMX[3F:4F] (doubles as the CE stream); rr = 1/s
                F = pg["F"]
                nc.scalar.activation(pg["PMX"][:, 3 * F : 4 * F], pg["st"][:],
                                     AF.Ln)
                nc.scalar.activation(pg["rr"][:], pg["PMX"][:, 3 * F : 4 * F],
                                     AF.Exp, scale=-1.0)


            def st_pe_cd(pg, plast):
                # streams C=(pm2,pm3), D=(pm1,lns) of group g
                F, lhs = pg["F"], pg["lhs"]
                pair_mm(1024, pg["PMX"][:, 0 : 2 * F], F, lhs,
                        pg["first"], plast)
                pair_mm(1536, pg["PMX"][:, 2 * F : 4 * F], F, lhs,
                        pg["first"], plast)

            def st_products(pg, ba_last, prev):
                # DVE products of group g; PE issue staggered so the PE
                # alternates (C,D)(g-1) then (B,A)(g) with every wait already
                # satisfied when its matmul reaches the queue head - the PE
                # stays gapless and ramps to the full 2.4 GHz p-state.
                F = pg["F"]
                PQX, PMX, MQ = pg["PQX"], pg["PMX"], pg["MQ"]
                nc.vector.tensor_tensor(
                    PQX[:, 2 * F : 3 * F], pg["ebig"][:, 2 * F : 3 * F],
                    pg["rr"][:], ALU.mult)
                nc.vector.tensor_tensor(
                    PQX[:, 3 * F : 4 * F], PQX[:, 2 * F : 3 * F],
                    MQ[:, 3 * F : 4 * F], ALU.mult)
                if prev is not None:
                    st_pe_cd(prev, False)
                pair_mm(512, PQX[:, 2 * F : 4 * F], F, pg["lhs"],
                        pg["first"], ba_last)
                for c in range(2):
                    nc.vector.tensor_tensor(
                        PQX[:, c * F : (c + 1) * F],
                        pg["ebig"][:, c * F : (c + 1) * F], pg["rr"][:],
                        ALU.mult)
                pair_mm(0, PQX[:, 0 : 2 * F], F, pg["lhs"],
                        pg["first"], ba_last)
                nc.vector.tensor_tensor(
                    PMX[:, 0 : 3 * F], PQX[:, 0 : 3 * F], MQ[:, 0 : 3 * F],
                    ALU.mult)

            def mk(g, GS, z0):
                F = GS * D
                lhs = {8: halves[:, 0:2], 4: halves[:, 2:4],
                       2: halves[:, 4:6]}[GS]
                return {
                    "F": F, "GS": GS, "z0": z0, "first": g == 0, "lhs": lhs,
                    "obig": inp.tile([128, 4 * F], dt.bfloat16, tag="obig",
                                     name="obig"),
                    "ebig": inp.tile([128, 4 * F], dt.bfloat16, tag="ebig",
                                     name="ebig"),
                    "MQ": minp.tile([128, 4 * F], dt.bfloat16, tag="MQ",
                                    name="MQ"),
                    "qa": work.tile([128, 2 * F], dt.bfloat16, tag="qa",
                                    name="qa"),
                    "st": work.tile([128, F], dt.bfloat16, tag="st", name="st"),
                    "rr": work.tile([128, F], dt.bfloat16, tag="rr", name="rr"),
                    "PQX": work3.tile([128, 4 * F], dt.bfloat16, tag="PQX",
                                      name="PQX"),
                    "PMX": work3.tile([128, 4 * F], dt.bfloat16, tag="PMX",
                                      name="PMX"),
                }

            G = len(GROUPS)
            gl = [None] * G
            z0 = 0
            for g, GS in enumerate(GROUPS):
                gl[g] = mk(g, GS, z0)
                z0 += GS
            st_dma_o(gl[0])
            st_dma_o(gl[1])
            st_dma_m(gl[0])
            nc.sync.dma_start(halves[:], hv_dram[:])
            # 2-deep software pipeline; per period k:
            #   ACT [exp(k), ln(k-1), rexp(k-1)]
            #   DVE [qa(k-1), st(k-1), products(k-2)]
            #   PE  [streams(k-2)] (interleaved with products)
            # No queue-head ever waits on work issued later in the same
            # period on another engine (except ln on st, which the 4F exp
            # covers).  o-DMAs run one group ahead of m-DMAs.
            for k in range(G + 2):
                if k + 2 < G:
                    st_dma_o(gl[k + 2])
                if k + 1 < G:
                    st_dma_m(gl[k + 1])
                if k < G:
                    st_exp(gl[k])
                if 1 <= k <= G:
                    st_sum(gl[k - 1])
                    st_act_ln(gl[k - 1])
                if 2 <= k <= G + 1:
                    g = k - 2
                    st_products(gl[g], g == G - 1,
                                gl[g - 1] if g >= 1 else None)
            st_pe_cd(gl[G - 1], True)

            # drain: psum -> sbuf split across two engines, then DMA out
            ob = outp.tile([2, 2048], dt.float32, tag="ob", name="ob")
            nc.vector.tensor_scalar(ob[:, 0:1024], ps[:, 0:1024], 0.0, None,
                                    mybir.AluOpType.add)
            nc.scalar.copy(ob[:, 1024:2048], ps[:, 1024:2048])
            nc.sync.dma_start(res_dram[:], ob[:])

    nc.compile()
    return nc


def _get_program():
    if "nc" not in _cache:
        _cache["nc"] = _build_program()
    return _cache["nc"]


def _is_structured(out, target, lbl, vor, n_cc):
    try:
        if int(n_cc) != NCC:
            return False
        if out.shape != (B, C, D, D, D) or target.shape != (B, 1, D, D, D):
            return False
        if lbl.shape != (B, D, D, D) or vor.shape != (B, D, D, D):
            return False
        bz = np.arange(D) // (D // 2)
        bx = np.arange(D) // (D // 4)
        grid = (bz[:, None, None] * 8 + bz[None, :, None] * 4 + bx[None, None, :] + 1)
        if not (vor == grid[None].astype(vor.dtype)).all():
            return False
        if not (lbl == np.where(target[:, 0] != 0, vor, 0).astype(lbl.dtype)).all():
            return False
        return True
    except Exception:
        return False


def _halves_np():
    import ml_dtypes

    hv = np.zeros((128, 6), dtype=ml_dtypes.bfloat16)
    p = np.arange(128)
    hv[(p % 16) < 8, 0] = 1
    hv[(p % 16) >= 8, 1] = 1
    hv[(p % 32) < 16, 2] = 1
    hv[(p % 32) >= 16, 3] = 1
    hv[(p % 64) < 32, 4] = 1
    hv[(p % 64) >= 32, 5] = 1
    return hv


def run_device(out, target, trace=False, trace_cores=None):
    """Run the 8-core device program; returns (per-core res arrays, results)."""
    import ml_dtypes
    from concourse.bass_utils import run_bass_kernel_spmd

    nc = _get_program()
    bf16 = ml_dtypes.bfloat16
    hv = _halves_np()
    # stage permuted-channel bf16 logits and one-hot masks once per sample
    operm = {}
    mperm = {}
    for b in range(B):
        operm[b] = np.ascontiguousarray(out[b, CPERM]).astype(bf16)
        oh = (target[b, 0, None] == np.array(CPERM)[:, None, None, None])
        mperm[b] = oh.astype(bf16)
    in_maps = []
    for i in range(NCORES):
        b, z0 = i // 4, ZSH * (i % 4)
        in_maps.append({
            "o": np.ascontiguousarray(operm[b][:, z0 : z0 + ZSH]),
            "m": np.ascontiguousarray(mperm[b][:, z0 : z0 + ZSH]),
            "hv": hv,
        })
    results = run_bass_kernel_spmd(
        nc, in_maps, core_ids=list(range(NCORES)), trace=trace,
        trace_cores=trace_cores,
    )
    return [results.results[i]["res"] for i in range(NCORES)], results


def _combine(res_list, out, target):
    """Host combine of per-core partial sums + exact target-derived counts."""
    N = D ** 3
    tgt = target[:, 0].astype(np.int64)

    cnt = np.zeros((B, C))
    fgb = np.zeros((B, 16))           # foreground voxels per Voronoi cell
    OTb = np.zeros((B, 16))           # exact block sums of o_tgt (f64, host)
    for b in range(B):
        cnt[b] = np.bincount(tgt[b].ravel(), minlength=C)[:C]
        fg = (tgt[b] != 0).reshape(2, 64, 2, 64, 4, 32)
        fgb[b] = fg.sum(axis=(1, 3, 5)).reshape(16)
        ot = np.take_along_axis(out[b], tgt[b][None], axis=0)[0].astype(np.float64)
        OTb[b] = ot.reshape(2, 64, 2, 64, 4, 32).sum(axis=(1, 3, 5)).reshape(16)

    P1 = np.zeros((B, 2, 2, 128))     # [b, bz, by, x] block partials of p1
    F1 = np.zeros((B, 2, 2, 128))     # ... of p1*(t==0)
    LS = np.zeros((B, 2, 2, 128))     # ... of ln(s)
    Sp = np.zeros((B, 3))             # global sums of p1, p2, p3
    tp = np.zeros((B, 3))             # global sums of pm1, pm2, pm3

    def fold(region):                 # [2, 256] -> [yhalf, x]
        return region.reshape(2, 2, 128).sum(axis=1)

    for i in range(NCORES):
        b, bz = i // 4, (i % 4) // 2
        r = res_list[i].astype(np.float64)
        Sp[b, 1] += r[:, 0:256].sum()
        Sp[b, 2] += r[:, 256:512].sum()
        P1[b, bz] += fold(r[:, 512:768])
        F1[b, bz] += fold(r[:, 768:1024])
        tp[b, 1] += r[:, 1024:1280].sum()
        tp[b, 2] += r[:, 1280:1536].sum()
        tp[b, 0] += r[:, 1536:1792].sum()
        LS[b, bz] += fold(r[:, 1792:2048])
    Sp[:, 0] = P1.sum(axis=(1, 2, 3))

    def blocks(arr):  # [b, bz, by, x] -> [b, 16] cells (bz*8 + by*4 + x//32)
        return arr.reshape(B, 2, 2, 4, 32).sum(axis=-1).reshape(B, 16)

    Pb, Fb, Lb = blocks(P1), blocks(F1), blocks(LS)

    # ---- global DC_and_CE (ce = lns - o_tgt) ----
    ce_global = (LS.sum() - OTb.sum()) / (B * N)
    fp = Sp - tp
    fn = cnt[:, 1:] - tp
    dc = (2.0 * tp + SMOOTH) / np.maximum(2.0 * tp + fp + fn + SMOOTH, 1e-8)
    global_loss = ce_global - dc.mean()

    # ---- per-component term ----
    cnt_block = float((D // 2) * (D // 2) * (D // 4))
    A = Pb - Fb                      # tp_c
    fn_c = fgb - A
    fp_c = Fb
    dc_c = (2.0 * A + SMOOTH) / np.maximum(2.0 * A + fn_c + fp_c + SMOOTH, 1e-8)
    ce_t = (Lb - OTb) / cnt_block
    cc_term = (-dc_c + ce_t).mean()

    return np.float32(global_loss + cc_term)


def _reference_numpy(out, target, lbl, vor, n_cc):
    """Exact fallback for arbitrary inputs (mirrors reference.py)."""
    n_cc = int(n_cc)
    o = out.astype(np.float64)
    tgt = target[:, 0].astype(np.int64)
    mx = o.max(axis=1, keepdims=True)
    eo = np.exp(o - mx)
    se = eo.sum(axis=1, keepdims=True)
    logp = o - mx - np.log(se)
    probs = np.exp(logp)
    ce_map = -np.take_along_axis(logp, tgt[:, None], axis=1)[:, 0]

    ce_global = ce_map.mean()
    onehot = (tgt[:, None] == np.arange(C)[None, :, None, None, None]).astype(np.float64)
    ax = (2, 3, 4)
    tp = (probs * onehot).sum(axis=ax)
    fp = (probs * (1.0 - onehot)).sum(axis=ax)
    fn = ((1.0 - probs) * onehot).sum(axis=ax)
    dc = (2.0 * tp + SMOOTH) / np.maximum(2.0 * tp + fp + fn + SMOOTH, 1e-8)
    dice_global = -dc[:, 1:].mean()
    global_loss = ce_global + dice_global

    p1 = probs[:, 1].reshape(B, -1)
    lblf = lbl.reshape(B, -1).astype(np.int64)
    vorf = vor.reshape(B, -1).astype(np.int64)
    cef = ce_map.reshape(B, -1)

    def seg(v, idx):
        outv = np.zeros((B, n_cc + 1))
        for b in range(B):
            outv[b] = np.bincount(idx[b], weights=v[b], minlength=n_cc + 1)[: n_cc + 1]
        return outv

    tp_c = seg(p1, lblf)[:, 1:]
    fn_c = seg(1.0 - p1, lblf)[:, 1:]
    fp_c = seg(p1 * (lblf == 0), vorf)[:, 1:]
    ce_c = seg(cef, vorf)[:, 1:]
    cnt_c = seg(np.ones_like(p1), vorf)[:, 1:]
    dc_c = (2.0 * tp_c + SMOOTH) / np.maximum(2.0 * tp_c + fn_c + fp_c + SMOOTH, 1e-8)
    ce_t = ce_c / np.maximum(cnt_c, 1.0)
    cc_term = (-dc_c + ce_t).mean()
    return np.float32(global_loss + cc_term)


def kernel(out, target, lbl, vor, n_cc):
    if not _is_structured(out, target, lbl, vor, n_cc):
        return _reference_numpy(out, target, lbl, vor, n_cc)
    res_list, _ = run_device(out, target)
    return _combine(res_list, out, target)


if __name__ == "__main__":
    rng = np.random.default_rng(0)
    o = rng.standard_normal((B, C, D, D, D), dtype=np.float32)
    t = rng.integers(0, C, (B, 1, D, D, D)).astype(np.int32)
    bz = np.arange(D) // (D // 2)
    bx = np.arange(D) // (D // 4)
    grid = (bz[:, None, None] * 8 + bz[None, :, None] * 4 + bx[None, None, :] + 1).astype(np.int32)
    v = np.broadcast_to(grid, (B, D, D, D)).copy()
    l = np.where(t[:, 0] != 0, v, 0).astype(np.int32)
    got = kernel(out=o, target=t, lbl=l, vor=v, n_cc=np.int64(16))
    want = _reference_numpy(o, t, l, v, 16)
    print("device:", got, "ref:", want, "rel err:", abs(got - want) / abs(want))
